# revision 1
# baseline (speedup 1.0000x reference)
"""Trainium2 Bass kernel for the controlled-U (CU) gate application.

Math: the reference builds U = P0 (x) I (x) ... + P1 (x) Mexp (x) I ...
with dim=2, wires=12, index=(0,1), control_state=(1,). This factors as

    U = diag(I_2, Mexp) (x) I_1024        (4096 x 4096)

so U @ x is:
    out[0:2048]     = x[0:2048]                        (identity)
    out[2048:3072]  = c00 * x[2048:3072] + c01 * x[3072:4096]
    out[3072:4096]  = c10 * x[2048:3072] + c11 * x[3072:4096]

with [[c00, c01], [c10, c11]] = Mexp = expm(M - M^H), a 2x2 unitary
computed exactly on host (eigendecomposition of the 2x2 Hermitian
generator).

Device strategy (8 NeuronCores, SPMD, row sharding — all DMA runs are
full 4 KiB rows):
  - core d gets top rows [256d, 256d+256) (identity) plus the bottom
    pair rows [2048+128d, +128) and [3072+128d, +128) (the 2x2 mix);
    every core runs the identical program on 1/8 of the work.
  - top rows: DVE strided copies interleave re/im -> complex64 layout
  - bottom pair rows are split by batch column between TensorE (fp32
    matmuls with 32x32 diagonal stationary tiles at concurrent tile
    positions, PSUM accumulation, ACT interleave-copies PSUM -> SBUF)
    and the DVE (fused scalar_tensor_tensor MAC chains with per-
    partition scalar coefficients, writing the interleaved layout
    directly), balanced so both engines hide under the DMA stream
  - outputs per core: f32 rows of interleaved (re, im) pairs; the host
    reassembles the (4096, 2048) f32 buffer and reinterprets it as
    complex64 (zero-copy view).

All arithmetic is fp32 (exact vs the reference up to rounding, ~1e-7).
"""

import numpy as np

import concourse.bacc as bacc
import concourse.mybir as mybir
from concourse.tile import TileContext
from concourse.bass_utils import run_bass_kernel_spmd

# Problem geometry (hardcoded per the task contract).
D = 4096           # state dimension 2**12
B = 1024           # batch
NCORES = 8
P = 128            # SBUF partitions
TROWS = D // 2 // NCORES   # 256 top (identity) rows per core
PROWS = D // 4 // NCORES   # 128 bottom pair rows per core
F32 = mybir.dt.float32

NDIAG = 12         # 12 diagonal coefficient matrices (see _coef_values)
TP = 32            # PE sub-tile size for tile_position concurrency
CH = B // 2        # column half processed per compute engine

# quantity -> (out half, interleave parity, coefficient idx per input).
# inputs are (xr1, xi1, xr2, xi2); coefficients include baked-in signs.
RECIPES = [
    ("o1re", 0, 0, (0, 1, 3, 4)),
    ("o1im", 0, 1, (2, 0, 5, 3)),
    ("o2re", 1, 0, (6, 7, 9, 10)),
    ("o2im", 1, 1, (8, 6, 11, 9)),
]


def _build_nc() -> bacc.Bacc:
    """Build the per-core Bass/Tile program (identical on all 8 cores)."""
    # Bacc (not raw Bass): its compile() lowers multi-dependency sync waits
    # through event semaphores — raw Bass trips walrus's per-instruction
    # wait-slot limit ("Too many sync wait commands").
    nc = bacc.Bacc("TRN2", enable_partition_id=False)

    xr_t = nc.dram_tensor("xr_t", [TROWS, B], F32, kind="ExternalInput")
    xi_t = nc.dram_tensor("xi_t", [TROWS, B], F32, kind="ExternalInput")
    xr_b1 = nc.dram_tensor("xr_b1", [PROWS, B], F32, kind="ExternalInput")
    xi_b1 = nc.dram_tensor("xi_b1", [PROWS, B], F32, kind="ExternalInput")
    xr_b2 = nc.dram_tensor("xr_b2", [PROWS, B], F32, kind="ExternalInput")
    xi_b2 = nc.dram_tensor("xi_b2", [PROWS, B], F32, kind="ExternalInput")
    # coef[p, k*TP + (p % TP)] = value_k  ->  32x32 diagonal blocks.
    coef = nc.dram_tensor("coef", [P, NDIAG * TP], F32, kind="ExternalInput")
    cvec = nc.dram_tensor("cvec", [P, NDIAG], F32, kind="ExternalInput")

    out_t = nc.dram_tensor("out_t", [TROWS, 2 * B], F32, kind="ExternalOutput")
    out_b1 = nc.dram_tensor("out_b1", [PROWS, 2 * B], F32, kind="ExternalOutput")
    out_b2 = nc.dram_tensor("out_b2", [PROWS, 2 * B], F32, kind="ExternalOutput")

    with TileContext(nc) as tc:
        with (
            tc.tile_pool(name="const", bufs=1) as const_pool,
            tc.tile_pool(name="io", bufs=3) as io_pool,
            tc.tile_pool(name="scr", bufs=2) as scr_pool,
            tc.tile_pool(name="psum", bufs=7, space="PSUM") as psum_pool,
            tc.tile_pool(name="psum_warm", bufs=1, space="PSUM") as warm_pool,
        ):
            # const loads go on the ACT ring (empty at start) so the sync
            # ring's first payload load issues immediately.
            coef_sb = const_pool.tile([P, NDIAG * TP], F32)
            nc.scalar.dma_start(coef_sb[:], coef[:])
            cvec_sb = const_pool.tile([P, NDIAG], F32)
            nc.scalar.dma_start(cvec_sb[:], cvec[:])

            def cdiag(k: int, i: int):
                """value_k * I_32 stationary for PE sub-tile row group i."""
                return coef_sb[i * TP : (i + 1) * TP, k * TP : (k + 1) * TP]

            def cval(k: int):
                """value_k as a per-partition scalar operand for the DVE."""
                return cvec_sb[:, k : k + 1]

            # Engine warmups: observe the small constant tiles with a cheap
            # op per engine, so no later instruction needs a multi-sem wait
            # (bacc funnels those through shared event semaphores, which can
            # serialize an engine behind unrelated work).
            warm_ps = warm_pool.tile([P, 2], F32, tag="warm")
            nc.tensor.matmul(warm_ps[:TP], cdiag(0, 0), coef_sb[:TP, 0:2],
                             start=True, stop=True, tile_position=(0, 0))
            warm_v = scr_pool.tile([P, 2], F32, tag="warm_v")
            nc.vector.tensor_copy(warm_v[:], cvec_sb[:, 0:2])

            # ---- bottom pair rows first: these gate PE/DVE compute ----
            # loads split by column half; half 0 (PE's data) lands first so
            # the TensorEngine starts as early as possible.
            b_in = {}
            srcs = (("r1", xr_b1), ("i1", xi_b1), ("r2", xr_b2), ("i2", xi_b2))
            for name, src in srcs:
                b_in[name] = io_pool.tile([P, B], F32, tag=name,
                                          name=f"bin_{name}")
            for c in range(2):
                cs = slice(c * CH, (c + 1) * CH)
                for name, src in srcs:
                    nc.sync.dma_start(b_in[name][:, cs], src[:, cs])

            o_b1 = io_pool.tile([P, 2 * B], F32, tag="o_b1")
            o_b2 = io_pool.tile([P, 2 * B], F32, tag="o_b2")
            o_b = {0: o_b1, 1: o_b2}

            def pe_mix(h, par, cks, cs: slice):
                """one output quantity over column range cs on the PE."""
                n = cs.stop - cs.start
                pt = psum_pool.tile([P, n], F32, tag="ps")
                movs = [b_in[nm][:, cs] for nm in ("r1", "i1", "r2", "i2")]
                for t, (k, mv) in enumerate(zip(cks, movs)):
                    # fp32 matmul costs 4 cyc/moving-column; the four 32x32
                    # diagonal sub-tiles at positions (32i, 32i) execute
                    # concurrently.
                    for i in range(P // TP):
                        nc.tensor.matmul(
                            pt[i * TP : (i + 1) * TP], cdiag(k, i),
                            mv[i * TP : (i + 1) * TP],
                            start=(t == 0), stop=(t == 3),
                            tile_position=(i * TP, i * TP),
                            skip_group_check=True,
                        )
                # ACT sits next to PSUM: interleave-copy PSUM -> SBUF
                nc.scalar.copy(
                    o_b[h][:, 2 * cs.start + par : 2 * cs.stop : 2], pt[:])

            def dve_mix(h, par, cks, cs: slice):
                """one output quantity over column range cs on the DVE."""
                ka, kb, kc, kd = cks
                n = cs.stop - cs.start
                r1 = b_in["r1"][:, cs]
                i1 = b_in["i1"][:, cs]
                r2 = b_in["r2"][:, cs]
                i2 = b_in["i2"][:, cs]
                mul = mybir.AluOpType.mult
                add = mybir.AluOpType.add
                t_a = scr_pool.tile([P, n], F32, tag="ta")
                t_b = scr_pool.tile([P, n], F32, tag="tb")
                # fused (in0 * scalar) + in1 chains: 4 ops per quantity
                nc.vector.tensor_scalar_mul(t_a[:], r1, cval(ka))
                nc.vector.scalar_tensor_tensor(
                    t_b[:], i1, cval(kb), t_a[:], mul, add)
                nc.vector.scalar_tensor_tensor(
                    t_a[:], r2, cval(kc), t_b[:], mul, add)
                nc.vector.scalar_tensor_tensor(
                    o_b[h][:, 2 * cs.start + par : 2 * cs.stop : 2],
                    i2, cval(kd), t_a[:], mul, add)

            c0 = slice(0, CH)
            c1 = slice(CH, B)
            for name, h, par, cks in RECIPES:   # all 4 quantities, half 0
                pe_mix(h, par, cks, c0)
            # o1re half 1 on the PE, psum split in two chunks so its ACT
            # interleave copies (and the dependent store) pipeline.
            pe_mix(*RECIPES[0][1:], slice(CH, CH + CH // 2))
            pe_mix(*RECIPES[0][1:], slice(CH + CH // 2, B))
            for name, h, par, cks in RECIPES[1:]:  # other 3, half 1 -> DVE
                dve_mix(h, par, cks, c1)

            # stores go on the ACT HWDGE ring: HWDGE is FIFO per issuing
            # engine, so a store waiting on compute must not block loads
            # (which are issued on the sync/SP ring and never wait).
            for h, dst in ((0, out_b1), (1, out_b2)):
                for c in range(2):
                    nc.scalar.dma_start(dst[:, c * B : (c + 1) * B],
                                        o_b[h][:, c * B : (c + 1) * B])

            # ---- top rows: identity, just interleave re/im ----
            for b in range(TROWS // P):
                rs = slice(b * P, (b + 1) * P)
                xr_g = io_pool.tile([P, B], F32, tag="xr_top")
                xi_g = io_pool.tile([P, B], F32, tag="xi_top")
                nc.sync.dma_start(xr_g[:], xr_t[rs, :])
                nc.sync.dma_start(xi_g[:], xi_t[rs, :])
                o_g = io_pool.tile([P, 2 * B], F32, tag="out_top")
                nc.vector.tensor_copy(o_g[:, 0 : 2 * B : 2], xr_g[:])
                nc.vector.tensor_copy(o_g[:, 1 : 2 * B : 2], xi_g[:])
                for c in range(2):
                    nc.scalar.dma_start(out_t[rs, c * B : (c + 1) * B],
                                        o_g[:, c * B : (c + 1) * B])

    nc.finalize()
    return nc


_NC_CACHE = None


def _get_nc() -> bacc.Bacc:
    global _NC_CACHE
    if _NC_CACHE is None:
        _NC_CACHE = _build_nc()
    return _NC_CACHE


def _coef_values(M_re: np.ndarray, M_im: np.ndarray):
    """Host-side 2x2 expm of the anti-Hermitian generator -> coef arrays."""
    M = M_re.astype(np.float64) + 1j * M_im.astype(np.float64)
    A = M - M.conj().T          # anti-Hermitian
    H = -1j * A                 # Hermitian
    w, V = np.linalg.eigh(H)
    Mexp = V @ np.diag(np.exp(1j * w)) @ V.conj().T   # expm(A), exact
    c00, c01 = Mexp[0, 0], Mexp[0, 1]
    c10, c11 = Mexp[1, 0], Mexp[1, 1]
    vals = [
        c00.real, -c00.imag, c00.imag,
        c01.real, -c01.imag, c01.imag,
        c10.real, -c10.imag, c10.imag,
        c11.real, -c11.imag, c11.imag,
    ]
    coef = np.zeros((P, NDIAG * TP), dtype=np.float32)
    idx = np.arange(P)
    for k, v in enumerate(vals):
        coef[idx, k * TP + (idx % TP)] = np.float32(v)
    cvec = np.tile(np.array(vals, dtype=np.float32), (P, 1))
    return coef, cvec


def _in_map(x_re, x_im, coef, cvec, d: int) -> dict:
    t0 = d * TROWS
    b1 = D // 2 + d * PROWS
    b2 = 3 * D // 4 + d * PROWS
    return {
        "xr_t": x_re[t0 : t0 + TROWS],
        "xi_t": x_im[t0 : t0 + TROWS],
        "xr_b1": x_re[b1 : b1 + PROWS],
        "xi_b1": x_im[b1 : b1 + PROWS],
        "xr_b2": x_re[b2 : b2 + PROWS],
        "xi_b2": x_im[b2 : b2 + PROWS],
        "coef": coef,
        "cvec": cvec,
    }


def kernel(M_re, M_im, x_re, x_im) -> np.ndarray:
    M_re = np.asarray(M_re, dtype=np.float32)
    M_im = np.asarray(M_im, dtype=np.float32)
    x_re = np.ascontiguousarray(x_re, dtype=np.float32)
    x_im = np.ascontiguousarray(x_im, dtype=np.float32)

    coef, cvec = _coef_values(M_re, M_im)
    in_maps = [_in_map(x_re, x_im, coef, cvec, d) for d in range(NCORES)]

    nc = _get_nc()
    res = run_bass_kernel_spmd(nc, in_maps, core_ids=list(range(NCORES)))

    full = np.empty((D, 2 * B), dtype=np.float32)
    for d, r in enumerate(res.results):
        t0 = d * TROWS
        b1 = D // 2 + d * PROWS
        b2 = 3 * D // 4 + d * PROWS
        full[t0 : t0 + TROWS] = r["out_t"]
        full[b1 : b1 + PROWS] = r["out_b1"]
        full[b2 : b2 + PROWS] = r["out_b2"]
    return full.view(np.complex64)  # (4096, 1024)



# revision 2
# speedup vs baseline: 1.7615x; 1.7615x over previous
"""Trainium2 Bass kernel for the controlled-U (CU) gate application.

Math: the reference builds U = P0 (x) I (x) ... + P1 (x) Mexp (x) I ...
with dim=2, wires=12, index=(0,1), control_state=(1,). This factors as

    U = diag(I_2048, Mexp (x) I_1024)        (4096 x 4096)

so U @ x is:
    out[0:2048]     = x[0:2048]                        (identity)
    out[2048:3072]  = c00 * x[2048:3072] + c01 * x[3072:4096]
    out[3072:4096]  = c10 * x[2048:3072] + c11 * x[3072:4096]

with [[c00, c01], [c10, c11]] = Mexp = expm(M - M^H), a 2x2 unitary
computed exactly on host (eigendecomposition of the 2x2 Hermitian
generator).

Device strategy (8 NeuronCores, SPMD, fp16 streaming; the rel-err
budget of 2e-2 dwarfs fp16's ~5e-4, so all payload traffic is 16-bit,
halving HBM bytes vs fp32):
  - core d owns top rows [256d, 256d+256) (identity) and the bottom
    pair rows [2048+128d, +128) / [3072+128d, +128).
  - the host packs per-core inputs into two [128, 4096] fp16 DRAM
    tensors. in_top is the identity payload; the kernel moves it
    HBM -> SBUF -> HBM untouched (no engine work). in_bot interleaves
    the four bottom planes (xr1, xi1, xr2, xi2) by 32-row groups so
    that one 128x128 stationary matrix W (16 diagonal 32x32 blocks
    holding the real 4x4 mix coefficients) turns each [128, 512]
    moving tile into all four output planes at once:
        out[32b+j, n] = sum_a G[b][a] * in[32a+j, n]
  - 8 fp16 matmuls (512 cols each, one PSUM bank each, 8 banks total),
    PSUM -> SBUF eviction with fp32->fp16 convert alternating between
    the ACT and DVE engines so both stay far below the DMA roofline.
  - loads ride the sync HWDGE ring, stores the ACT HWDGE ring; every
    transfer is >=512 KiB with 4 KiB/partition descriptors.
  - outputs are fp16; the host upcasts and reassembles the complex64
    result (gather/unshard), which does not touch device time.
"""

import numpy as np

import concourse.bacc as bacc
import concourse.mybir as mybir
from concourse.tile import TileContext
from concourse.bass_utils import run_bass_kernel_spmd

# Problem geometry (hardcoded per the task contract).
D = 4096           # state dimension 2**12
B = 1024           # batch
NCORES = 8
P = 128            # SBUF partitions
TROWS = D // 2 // NCORES   # 256 top (identity) rows per core
PROWS = D // 4 // NCORES   # 128 bottom pair rows per core
F16 = mybir.dt.float16
F32 = mybir.dt.float32

NCOL = 4 * B       # 4096 packed columns per [128, NCOL] payload tensor
MMCOL = 512        # moving columns per matmul (= one PSUM bank of fp32)
NMM = NCOL // MMCOL


def _build_nc() -> bacc.Bacc:
    """Build the per-core Bass/Tile program (identical on all 8 cores)."""
    # Bacc (not raw Bass): its compile() lowers multi-dependency sync waits
    # through event semaphores — raw Bass trips walrus's per-instruction
    # wait-slot limit ("Too many sync wait commands").
    nc = bacc.Bacc("TRN2", enable_partition_id=False)

    in_top = nc.dram_tensor("in_top", [P, NCOL], F16, kind="ExternalInput")
    in_bot = nc.dram_tensor("in_bot", [P, NCOL], F16, kind="ExternalInput")
    wmat = nc.dram_tensor("wmat", [P, P], F16, kind="ExternalInput")

    out_top = nc.dram_tensor("out_top", [P, NCOL], F16, kind="ExternalOutput")
    out_bot = nc.dram_tensor("out_bot", [P, NCOL], F16, kind="ExternalOutput")

    with TileContext(nc) as tc:
        with (
            tc.tile_pool(name="const", bufs=1) as const_pool,
            tc.tile_pool(name="io", bufs=1) as io_pool,
            tc.tile_pool(name="psum", bufs=NMM, space="PSUM") as psum_pool,
        ):
            # stationary mix matrix on the ACT ring (empty at start) so the
            # sync ring's first payload load issues immediately.
            w_sb = const_pool.tile([P, P], F16)
            nc.scalar.dma_start(w_sb[:], wmat[:])

            t_bot = io_pool.tile([P, NCOL], F16, tag="t_bot")
            t_top = io_pool.tile([P, NCOL], F16, tag="t_top")
            t_out = io_pool.tile([P, NCOL], F16, tag="t_out")

            # payload loads, sync ring: bottom first (it gates compute),
            # then the identity passthrough.
            half = NCOL // 2
            for c in range(2):
                cs = slice(c * half, (c + 1) * half)
                nc.sync.dma_start(t_bot[:, cs], in_bot[:, cs])
            for c in range(2):
                cs = slice(c * half, (c + 1) * half)
                nc.sync.dma_start(t_top[:, cs], in_top[:, cs])

            # 8 matmuls; each fills one PSUM bank with all 4 output planes
            # for one 32-row x 512-col chunk. Eviction alternates ACT/DVE.
            for h in range(NMM):
                hs = slice(h * MMCOL, (h + 1) * MMCOL)
                ps = psum_pool.tile([P, MMCOL], F32, tag="ps")
                nc.tensor.matmul(ps[:], w_sb[:], t_bot[:, hs],
                                 start=True, stop=True)
                if h % 2 == 0:
                    nc.scalar.copy(t_out[:, hs], ps[:])
                else:
                    nc.vector.tensor_copy(t_out[:, hs], ps[:])
                if h % 4 == 3:
                    # store the finished 2048-col half on the ACT ring
                    ss = slice((h - 3) * MMCOL, (h + 1) * MMCOL)
                    nc.scalar.dma_start(out_bot[:, ss], t_out[:, ss])

            # identity passthrough stores (depend only on their loads)
            for c in range(2):
                cs = slice(c * half, (c + 1) * half)
                nc.scalar.dma_start(out_top[:, cs], t_top[:, cs])

    nc.finalize()
    return nc


_NC_CACHE = None


def _get_nc() -> bacc.Bacc:
    global _NC_CACHE
    if _NC_CACHE is None:
        _NC_CACHE = _build_nc()
    return _NC_CACHE


def _mix_matrix(M_re: np.ndarray, M_im: np.ndarray) -> np.ndarray:
    """Host-side 2x2 expm of the anti-Hermitian generator -> 128x128 fp16
    stationary matrix W with W[32a+j, 32b+j] = G[b][a] (matmul computes
    W.T @ moving, i.e. out[32b+j] = sum_a G[b][a] * in[32a+j])."""
    M = M_re.astype(np.float64) + 1j * M_im.astype(np.float64)
    A = M - M.conj().T          # anti-Hermitian
    H = -1j * A                 # Hermitian
    w, V = np.linalg.eigh(H)
    Mexp = V @ np.diag(np.exp(1j * w)) @ V.conj().T   # expm(A), exact
    c00, c01 = Mexp[0, 0], Mexp[0, 1]
    c10, c11 = Mexp[1, 0], Mexp[1, 1]
    G = np.array([
        [c00.real, -c00.imag, c01.real, -c01.imag],
        [c00.imag,  c00.real, c01.imag,  c01.real],
        [c10.real, -c10.imag, c11.real, -c11.imag],
        [c10.imag,  c10.real, c11.imag,  c11.real],
    ])
    W = np.zeros((P, P), dtype=np.float32)
    j = np.arange(32)
    for a in range(4):
        for b in range(4):
            W[32 * a + j, 32 * b + j] = G[b, a]
    return W.astype(np.float16)


def _build_in_maps(M_re, M_im, x_re, x_im) -> list[dict]:
    """fp16-quantize + pack the full inputs into per-core tensors."""
    W = _mix_matrix(np.asarray(M_re, np.float32), np.asarray(M_im, np.float32))
    xr = np.asarray(x_re, np.float32).astype(np.float16)
    xi = np.asarray(x_im, np.float32).astype(np.float16)

    in_maps = []
    for d in range(NCORES):
        t0 = d * TROWS
        b1 = D // 2 + d * PROWS
        b2 = 3 * D // 4 + d * PROWS
        in_top = np.empty((P, NCOL), np.float16)
        in_top[:, : NCOL // 2] = (
            xr[t0 : t0 + TROWS].reshape(2, P, B).transpose(1, 0, 2).reshape(P, 2 * B)
        )
        in_top[:, NCOL // 2 :] = (
            xi[t0 : t0 + TROWS].reshape(2, P, B).transpose(1, 0, 2).reshape(P, 2 * B)
        )
        planes = np.stack([
            xr[b1 : b1 + PROWS], xi[b1 : b1 + PROWS],
            xr[b2 : b2 + PROWS], xi[b2 : b2 + PROWS],
        ])  # [4, 128, 1024]
        in_bot = np.ascontiguousarray(
            planes.reshape(4, 4, 32, B).transpose(0, 2, 1, 3).reshape(P, NCOL)
        )
        in_maps.append({"in_top": in_top, "in_bot": in_bot, "wmat": W})
    return in_maps


def _assemble(results: list[dict]) -> np.ndarray:
    """Gather/unshard: upcast fp16 per-core outputs into the complex64
    full-shape result."""
    full = np.empty((D, B), dtype=np.complex64)
    for d, r in enumerate(results):
        t0 = d * TROWS
        b1 = D // 2 + d * PROWS
        b2 = 3 * D // 4 + d * PROWS
        ot = r["out_top"]
        full.real[t0 : t0 + TROWS] = (
            ot[:, : NCOL // 2].reshape(P, 2, B).transpose(1, 0, 2).reshape(TROWS, B)
        )
        full.imag[t0 : t0 + TROWS] = (
            ot[:, NCOL // 2 :].reshape(P, 2, B).transpose(1, 0, 2).reshape(TROWS, B)
        )
        ob = (
            r["out_bot"].reshape(4, 32, 4, B).transpose(0, 2, 1, 3).reshape(4, PROWS, B)
        )
        full.real[b1 : b1 + PROWS] = ob[0]
        full.imag[b1 : b1 + PROWS] = ob[1]
        full.real[b2 : b2 + PROWS] = ob[2]
        full.imag[b2 : b2 + PROWS] = ob[3]
    return full


def kernel(M_re, M_im, x_re, x_im) -> np.ndarray:
    in_maps = _build_in_maps(M_re, M_im, x_re, x_im)
    nc = _get_nc()
    res = run_bass_kernel_spmd(nc, in_maps, core_ids=list(range(NCORES)))
    return _assemble(res.results)  # (4096, 1024) complex64
